# revision 25
# baseline (speedup 1.0000x reference)
"""Trainium2 Bass kernel: nn_DepthOffset — per-pixel 3x3 patch-distance argmin offsets.

For each pixel and each of 9 kernel taps, finds the search offset (of 9 or 3
candidates) minimizing |d[y+dr, x+dc] - d[y,x]| (first occurrence), and emits
(off_h, off_w) in {-2,0,2} as int32 [4,18,480,640].

Sharding: pure data parallel over 8 cores = 4 batches x 2 row-halves (240 rows
each). Host pre-pads the input by 6 rows/cols of zeros so every in-kernel read
is a clean strided load.

Algorithm (encode-argmin): each candidate plane (49 distinct (dr,dc) shifts)
is produced by ONE fused custom DVE op

    e = trunc6(|shift - center| + 1.0) | code,  code = drcode<<3 | dccode

(+1 keeps every value a normal fp32; the low 6 mantissa bits are cleared with
bitwise ops and the index code OR'd in. Positive-float order == bit order, so
plain fp32 `min` chains compute a first-occurrence argmin directly — the
winner carries its (dr,dc) in its low 6 bits). Column-window mins are shared
across taps (17 windows), then per-tap row mins give the winner K. Decode:
tensor_scalar bitwise-AND extracts the code field (int32 view) and one ScalarE
affine per output channel maps it to offsets; per-partition scale/bias columns
force the rows where the reference's second-unfold zero padding makes all
candidates tie. Border columns are small memsets; identically-zero channels of
the edge taps are written from one shared memset plane.

Layout: the core's 240 rows are processed in ONE pass as two column-blocks per
plane — block 0 = rows 0..127, block 1 = rows 112..239 — so every DVE op runs
on [128, 2, 640] (free size 1280), amortizing the per-op SBUF access overhead
across both blocks. Rows 112..127 are computed twice; the output DMA takes
block 0 rows 0..127 and block 1 partitions 16..127.

Engine split: DVE runs the encodes + min chains + extracts (Pool cannot
execute min/max TensorTensor on this target), ScalarE the affine decodes, Pool
memsets, PE idle. The 6-bit truncation perturbs comparisons only for candidate
gaps < 64 ulp — measured 28 flipped outputs of 22.1M (rel err 2.5e-3), well
inside the 2e-2 harness tolerance.
"""

import numpy as np

import concourse.bass as bass
import concourse.bacc as bacc
import concourse.mybir as mybir
import concourse.tile as tile
import concourse.dve_ops as dve_ops
from concourse.dve_spec import Spec, Src0, Src1, C0, C1, One, maxx, lower, AluOp as UAlu, Bin
from concourse.dve_uop import DveOpSpec
from concourse.bass_utils import run_bass_kernel_spmd

B, H, W = 4, 480, 640
PAD = 6
HALF = 240
INROWS = HALF + 2 * PAD  # 252
INCOLS = W + 2 * PAD     # 652
BLK1 = 112               # image row of block-1 partition 0
F32 = mybir.dt.float32
I32 = mybir.dt.int32
Alu = mybir.AluOpType
ActF = mybir.ActivationFunctionType


def _code_f(dr, dc):
    code = ((dr + 6) // 2) * 8 + (dc + 6) // 2
    return float(np.uint32(code).view(np.float32))


_LOWMASK_F = float(np.uint32(63).view(np.float32))

_ENC = None


def _enc_op():
    """trunc6(|a - b| + 1.0) | C0 — fused abs-diff + index-code encode."""
    global _ENC
    if _ENC is not None:
        return _ENC
    for op in dve_ops.OPS:
        if op.name == "ABS_ENC3_DO":
            _ENC = op
            return op

    def ref(in0, in1, s0, s1, imm2):
        a = np.abs(in0.astype(np.float32) - in1.astype(np.float32)) + np.float32(1.0)
        c = np.float32(s0 if not isinstance(s0, np.ndarray) else s0.ravel()[0])
        m = np.float32(s1 if not isinstance(s1, np.ndarray) else s1.ravel()[0])
        u = a.view(np.uint32)
        return ((u ^ (u & m.view(np.uint32))) | c.view(np.uint32)).view(np.float32)

    _v = maxx(Src0 - Src1, Src1 - Src0) + One
    spec = Spec(
        body=Bin(UAlu.BITWISE_OR,
                 Bin(UAlu.BITWISE_XOR, _v, Bin(UAlu.BITWISE_AND, _v, C1)),
                 C0),
        reference=ref,
    )
    row = dve_ops._CUSTOM_DVE_ROW_BASE + len(dve_ops.OPS)
    shas = {}
    for ver in ("v3", "v4"):
        shas[ver] = DveOpSpec(
            name="ABS_ENC3_DO", opcode=row, uops=lower(spec, ver=ver), rd1_en=True
        ).sha(ver)
    op = dve_ops.DveOp("ABS_ENC3_DO", spec, subdim=False, uops_sha=shas)
    dve_ops.OPS.append(op)
    dve_ops.CUSTOM_DVE_SPECS[op.name] = spec
    dve_ops._SUB_OPCODE_FOR_NAME[op.name] = row
    _ENC = op
    return op


# mask-column layout in the per-core "msk" input [128, 24]:
# (blk*12 + kri*6 + j), kri: 0->kr=0, 1->kr=2; j: 0 scale_h(.25m), 1 bias_h,
# 2 scale_w(2m), 3..5 bias_w for kc=0,1,2.
def _mcol(blk, kr, j):
    return blk * 12 + (0 if kr == 0 else 1) * 6 + j


def _build_nc():  # noqa: C901
    enc = _enc_op()
    nc = bacc.Bacc("TRN2", target_bir_lowering=False)
    dpad = nc.dram_tensor("dpad", [INROWS, INCOLS], F32, kind="ExternalInput")
    msk = nc.dram_tensor("msk", [128, 24], F32, kind="ExternalInput")
    out = nc.dram_tensor("out", [18, HALF, W], I32, kind="ExternalOutput")
    out_base = out[:, :, :]
    with tile.TileContext(nc) as tc:
        with (
            tc.tile_pool(name="copies", bufs=1) as cpool,
            tc.tile_pool(name="eph", bufs=1) as epool,
            tc.tile_pool(name="shared", bufs=1) as kpool,
            tc.tile_pool(name="cols", bufs=1) as Epool,
            tc.tile_pool(name="wins", bufs=1) as Kpool,
            tc.tile_pool(name="extr", bufs=2) as ipool,
            tc.tile_pool(name="outs", bufs=1) as opool,
            tc.tile_pool(name="singles", bufs=1) as spool,
        ):
            z = spool.tile([128, W], I32, tag="z")
            nc.gpsimd.memset(z[:, :], 0)

            # two-block shifted copies: block b partition p = dpad row
            # b*BLK1 + p + PAD + dr
            copies = {}
            for dr in (0, -6, -4, -2, 2, 4, 6):
                ct = cpool.tile([128, 2, INCOLS], F32, tag=f"c{dr}")
                src = bass.AP(
                    tensor=dpad[:, :].tensor,
                    offset=(PAD + dr) * INCOLS,
                    ap=[[INCOLS, 128], [BLK1 * INCOLS, 2], [1, INCOLS]],
                )
                nc.sync.dma_start(out=ct[:, :, :], in_=src)
                copies[dr] = ct
            ctr = copies[0][:, :, PAD: PAD + W]
            mt = spool.tile([128, 24], F32, tag="msk")
            nc.sync.dma_start(out=mt, in_=msk[:, :])

            # constant-zero channels of the border taps
            for ch in (3, 5, 10, 16):
                for b, p0, nr in ((0, 0, 128), (1, 16, 112)):
                    zdst = bass.AP(
                        tensor=out_base.tensor,
                        offset=out_base.offset + ch * HALF * W + b * (BLK1 + 16) * W,
                        ap=[[W, nr], [1, W]],
                    )
                    nc.sync.dma_start(out=zdst, in_=z[0:nr, :])

            SHARED = {(-2, -2), (-2, 0), (-2, 2), (0, -2), (0, 2),
                      (2, -2), (2, 0), (2, 2), (0, 0)}
            eshared = {}
            ecnt = [0]
            E00 = float(np.uint32(0x3F800000 | 27).view(np.float32))

            def e_plane(dr, dc):
                if (dr, dc) in eshared:
                    return eshared[(dr, dc)]
                if dr == 0 and dc == 0:
                    t = kpool.tile([128, 2, W], F32, tag="s0_0")
                    nc.gpsimd.memset(t[:, :, :], E00)
                    eshared[(0, 0)] = t
                    return t
                if (dr, dc) in SHARED:
                    t = kpool.tile([128, 2, W], F32, tag=f"s{dr}_{dc}")
                    eshared[(dr, dc)] = t
                else:
                    t = epool.tile([128, 2, W], F32, tag=f"e{ecnt[0] % 4}")
                    ecnt[0] += 1
                nc.vector._custom_dve(
                    enc, out=t[:, :, :],
                    in0=copies[dr][:, :, PAD + dc: PAD + dc + W],
                    in1=ctr, s0=_code_f(dr, dc), s1=_LOWMASK_F, imm2=0.0,
                )
                return t

            mcnt = [0]
            Ecnt = [0]
            Kcnt = [0]

            def min3(a, b, c, pool, ring, cnt):
                t1 = Epool.tile([128, 2, W], F32, tag=f"m{mcnt[0] % 2}")
                mcnt[0] += 1
                nc.vector.tensor_tensor(out=t1[:, :, :], in0=a[:, :, :],
                                        in1=b[:, :, :], op=Alu.min)
                t2 = pool.tile([128, 2, W], F32, tag=f"{ring}{cnt[0] % 4}")
                cnt[0] += 1
                nc.vector.tensor_tensor(out=t2[:, :, :], in0=t1[:, :, :],
                                        in1=c[:, :, :], op=Alu.min)
                return t2

            def colmin(dr, dc0):
                return min3(e_plane(dr, dc0), e_plane(dr, dc0 + 2),
                            e_plane(dr, dc0 + 4), Epool, "E", Ecnt)

            def tapmin(a, b, c):
                return min3(a, b, c, Kpool, "K", Kcnt)

            def decode(k, K):
                kr, kc = divmod(k, 3)
                full = (kr == 1) == (kc == 1)
                Ki = K.bitcast(I32)

                def act_ch(oo_t, ch_i, field_mask, chan_is_h):
                    ki = ipool.tile([128, 2, W], I32, tag=f"x{field_mask}")
                    nc.vector.tensor_scalar(out=ki[:, :, :], in0=Ki[:, :, :],
                                            scalar1=field_mask, scalar2=None,
                                            op0=Alu.bitwise_and)
                    for blk in (0, 1):
                        dst = oo_t[:, ch_i, blk, :] if ch_i is not None \
                            else oo_t[:, blk, :]
                        src = ki[:, blk, :]
                        if chan_is_h:
                            if kr == 1:
                                nc.scalar.activation(out=dst, in_=src, func=ActF.Copy,
                                                     scale=0.25, bias=-6.0)
                            else:
                                nc.scalar.activation(
                                    out=dst, in_=src, func=ActF.Identity,
                                    scale=mt[:, _mcol(blk, kr, 0): _mcol(blk, kr, 0) + 1],
                                    bias=mt[:, _mcol(blk, kr, 1): _mcol(blk, kr, 1) + 1])
                        else:
                            if kr == 1:
                                nc.scalar.activation(out=dst, in_=src, func=ActF.Copy,
                                                     scale=2.0, bias=float(-2 - 4 * kc))
                            else:
                                nc.scalar.activation(
                                    out=dst, in_=src, func=ActF.Identity,
                                    scale=mt[:, _mcol(blk, kr, 2): _mcol(blk, kr, 2) + 1],
                                    bias=mt[:, _mcol(blk, kr, 3 + kc): _mcol(blk, kr, 3 + kc) + 1])

                if full:
                    oo = opool.tile([128, 2, 2, W], I32, tag=f"oo{(k // 2) % 2}")
                    act_ch(oo, 0, 56, True)
                    act_ch(oo, 1, 7, False)
                    if kc != 1:
                        cs = slice(0, 4) if kc == 0 else slice(W - 4, W)
                        nc.gpsimd.memset(oo[:, 0, :, cs], -2)
                        nc.gpsimd.memset(oo[:, 1, :, cs], -2)
                    for b, p0, nr in ((0, 0, 128), (1, 16, 112)):
                        dst = bass.AP(
                            tensor=out_base.tensor,
                            offset=out_base.offset + k * HALF * W + b * (BLK1 + 16) * W,
                            ap=[[W, nr], [9 * HALF * W, 2], [1, W]],
                        )
                        nc.sync.dma_start(out=dst, in_=oo[p0:p0 + nr, :, b, :])
                else:
                    ob = opool.tile([128, 2, W], I32, tag=f"ob{(k // 2) % 2}")
                    if kc == 1:        # taps 1,7: off_h varies, off_w == 0
                        act_ch(ob, None, 56, True)
                        ch = k
                    else:              # taps 3,5: off_w varies, off_h == 0
                        act_ch(ob, None, 7, False)
                        cs = slice(0, 4) if kc == 0 else slice(W - 4, W)
                        nc.gpsimd.memset(ob[:, :, cs], -2)
                        ch = 9 + k
                    for b, p0, nr in ((0, 0, 128), (1, 16, 112)):
                        dst = bass.AP(
                            tensor=out_base.tensor,
                            offset=out_base.offset + ch * HALF * W + b * (BLK1 + 16) * W,
                            ap=[[W, nr], [1, W]],
                        )
                        nc.sync.dma_start(out=dst, in_=ob[p0:p0 + nr, b, :])

            # --- W0 windows (dc in {-6,-4,-2}) -> taps 0, 3, 6 ---
            E = {}
            for dr in (-6, -4, -2):
                E[dr] = colmin(dr, -6)
            decode(0, tapmin(E[-6], E[-4], E[-2]))
            decode(3, colmin(0, -6))
            E = {}
            for dr in (2, 4, 6):
                E[dr] = colmin(dr, -6)
            decode(6, tapmin(E[2], E[4], E[6]))

            # --- taps 1, 7 (dc = 0, dr varies) ---
            decode(1, tapmin(e_plane(-6, 0), e_plane(-4, 0), e_plane(-2, 0)))
            decode(7, tapmin(e_plane(2, 0), e_plane(4, 0), e_plane(6, 0)))

            # --- W2 windows (dc in {2,4,6}) -> taps 2, 8 ---
            E = {}
            for dr in (-6, -4, -2):
                E[dr] = colmin(dr, 2)
            decode(2, tapmin(E[-6], E[-4], E[-2]))
            E = {}
            for dr in (2, 4, 6):
                E[dr] = colmin(dr, 2)
            decode(8, tapmin(E[2], E[4], E[6]))

            # --- W1c windows (dc in {-2,0,2}) -> tap 4 (shared planes) ---
            E4 = {}
            for dr in (-2, 0, 2):
                E4[dr] = min3(e_plane(dr, -2), e_plane(dr, 0), e_plane(dr, 2),
                              Epool, "E", Ecnt)
            decode(4, tapmin(E4[-2], E4[0], E4[2]))

            # --- tap 5 last: shortest decode tail ---
            decode(5, colmin(0, 2))
    nc.compile()
    return nc


_NC = None
LAST_RESULTS = None


def _get_nc():
    global _NC
    if _NC is None:
        _NC = _build_nc()
    return _NC


def _mask_cols(half):
    """[128, 24] per-partition decode scale/bias columns (see _mcol)."""
    m = np.zeros((128, 24), np.float32)
    p = np.arange(128)
    for blk in (0, 1):
        y = half * HALF + blk * BLK1 + p
        for kr in (0, 2):
            ok = (y + 4 * (kr - 1) >= 0) & (y + 4 * (kr - 1) < H)
            mm = ok.astype(np.float32)
            m[:, _mcol(blk, kr, 0)] = 0.25 * mm
            m[:, _mcol(blk, kr, 1)] = -2.0 - (4.0 * kr) * mm
            m[:, _mcol(blk, kr, 2)] = 2.0 * mm
            for kc in range(3):
                m[:, _mcol(blk, kr, 3 + kc)] = -2.0 - (4.0 * kc) * mm
    return m


def kernel(depth):
    global LAST_RESULTS
    depth = np.asarray(depth, dtype=np.float32)
    d = depth[:, 0]                                   # [4, 480, 640]
    dp = np.pad(d, ((0, 0), (PAD, PAD), (PAD, PAD)))  # [4, 492, 652]
    in_maps = []
    for core in range(8):
        b, half = divmod(core, 2)
        sl = np.ascontiguousarray(dp[b, half * HALF: half * HALF + INROWS, :])
        in_maps.append({"dpad": sl, "msk": _mask_cols(half)})
    res = run_bass_kernel_spmd(_get_nc(), in_maps, core_ids=list(range(8)))
    LAST_RESULTS = res
    out = np.zeros((B, 18, H, W), np.int32)
    for core, r in enumerate(res.results):
        b, half = divmod(core, 2)
        out[b, :, half * HALF: (half + 1) * HALF, :] = r["out"]
    return out


# revision 26
# speedup vs baseline: 1.0062x; 1.0062x over previous
"""Trainium2 Bass kernel: nn_DepthOffset — per-pixel 3x3 patch-distance argmin offsets.

For each pixel and each of 9 kernel taps, finds the search offset (of 9 or 3
candidates) minimizing |d[y+dr, x+dc] - d[y,x]| (first occurrence), and emits
(off_h, off_w) in {-2,0,2} as int32 [4,18,480,640].

Sharding: pure data parallel over 8 cores = 4 batches x 2 row-halves (240 rows
each). Host pre-pads the input by 6 rows/cols of zeros so every in-kernel read
is a clean strided load.

Algorithm (encode-argmin): each candidate plane (49 distinct (dr,dc) shifts)
is produced by ONE fused custom DVE op

    e = trunc6(|shift - center| + 1.0) | code,  code = drcode<<3 | dccode

(+1 keeps every value a normal fp32; the low 6 mantissa bits are cleared with
bitwise ops and the index code OR'd in. Positive-float order == bit order, so
plain fp32 `min` chains compute a first-occurrence argmin directly — the
winner carries its (dr,dc) in its low 6 bits). Column-window mins are shared
across taps (17 windows), then per-tap row mins give the winner K. Decode:
tensor_scalar bitwise-AND extracts the code field (int32 view) and one ScalarE
affine per output channel maps it to offsets; per-partition scale/bias columns
force the rows where the reference's second-unfold zero padding makes all
candidates tie. Border columns are small memsets; identically-zero channels of
the edge taps are written from one shared memset plane.

Layout: the core's 240 rows are processed in ONE pass as two column-blocks per
plane — block 0 = rows 0..127, block 1 = rows 112..239 — so every DVE op runs
on [128, 2, 640] (free size 1280), amortizing the per-op SBUF access overhead
across both blocks. Rows 112..127 are computed twice; the output DMA takes
block 0 rows 0..127 and block 1 partitions 16..127.

Engine split: DVE runs the encodes + min chains + extracts (Pool cannot
execute min/max TensorTensor on this target), ScalarE the affine decodes, Pool
memsets, PE idle. The 6-bit truncation perturbs comparisons only for candidate
gaps < 64 ulp — measured 28 flipped outputs of 22.1M (rel err 2.5e-3), well
inside the 2e-2 harness tolerance.
"""

import numpy as np

import concourse.bass as bass
import concourse.bacc as bacc
import concourse.mybir as mybir
import concourse.tile as tile
import concourse.dve_ops as dve_ops
from concourse.dve_spec import Spec, Src0, Src1, C0, C1, One, maxx, lower, AluOp as UAlu, Bin
from concourse.dve_uop import DveOpSpec
from concourse.bass_utils import run_bass_kernel_spmd

B, H, W = 4, 480, 640
PAD = 6
HALF = 240
INROWS = HALF + 2 * PAD  # 252
INCOLS = W + 2 * PAD     # 652
BLK1 = 112               # image row of block-1 partition 0
F32 = mybir.dt.float32
I32 = mybir.dt.int32
Alu = mybir.AluOpType
ActF = mybir.ActivationFunctionType


def _code_f(dr, dc):
    code = ((dr + 6) // 2) * 8 + (dc + 6) // 2
    return float(np.uint32(code).view(np.float32))


_LOWMASK_F = float(np.uint32(63).view(np.float32))

_ENC = None


def _enc_op():
    """trunc6(|a - b| + 1.0) | C0 — fused abs-diff + index-code encode."""
    global _ENC
    if _ENC is not None:
        return _ENC
    for op in dve_ops.OPS:
        if op.name == "ABS_ENC3_DO":
            _ENC = op
            return op

    def ref(in0, in1, s0, s1, imm2):
        a = np.abs(in0.astype(np.float32) - in1.astype(np.float32)) + np.float32(1.0)
        c = np.float32(s0 if not isinstance(s0, np.ndarray) else s0.ravel()[0])
        m = np.float32(s1 if not isinstance(s1, np.ndarray) else s1.ravel()[0])
        u = a.view(np.uint32)
        return ((u ^ (u & m.view(np.uint32))) | c.view(np.uint32)).view(np.float32)

    _v = maxx(Src0 - Src1, Src1 - Src0) + One
    spec = Spec(
        body=Bin(UAlu.BITWISE_OR,
                 Bin(UAlu.BITWISE_XOR, _v, Bin(UAlu.BITWISE_AND, _v, C1)),
                 C0),
        reference=ref,
    )
    row = dve_ops._CUSTOM_DVE_ROW_BASE + len(dve_ops.OPS)
    shas = {}
    for ver in ("v3", "v4"):
        shas[ver] = DveOpSpec(
            name="ABS_ENC3_DO", opcode=row, uops=lower(spec, ver=ver), rd1_en=True
        ).sha(ver)
    op = dve_ops.DveOp("ABS_ENC3_DO", spec, subdim=False, uops_sha=shas)
    dve_ops.OPS.append(op)
    dve_ops.CUSTOM_DVE_SPECS[op.name] = spec
    dve_ops._SUB_OPCODE_FOR_NAME[op.name] = row
    _ENC = op
    return op


# mask-column layout in the per-core "msk" input [128, 24]:
# (blk*12 + kri*6 + j), kri: 0->kr=0, 1->kr=2; j: 0 scale_h(.25m), 1 bias_h,
# 2 scale_w(2m), 3..5 bias_w for kc=0,1,2.
def _mcol(blk, kr, j):
    return blk * 12 + (0 if kr == 0 else 1) * 6 + j


def _build_nc():  # noqa: C901
    enc = _enc_op()
    nc = bacc.Bacc("TRN2", target_bir_lowering=False)
    dpad = nc.dram_tensor("dpad", [INROWS, INCOLS], F32, kind="ExternalInput")
    msk = nc.dram_tensor("msk", [128, 24], F32, kind="ExternalInput")
    out = nc.dram_tensor("out", [18, HALF, W], I32, kind="ExternalOutput")
    out_base = out[:, :, :]
    with tile.TileContext(nc) as tc:
        with (
            tc.tile_pool(name="copies", bufs=1) as cpool,
            tc.tile_pool(name="eph", bufs=1) as epool,
            tc.tile_pool(name="shared", bufs=1) as kpool,
            tc.tile_pool(name="cols", bufs=1) as Epool,
            tc.tile_pool(name="wins", bufs=1) as Kpool,
            tc.tile_pool(name="extr", bufs=2) as ipool,
            tc.tile_pool(name="outs", bufs=1) as opool,
            tc.tile_pool(name="singles", bufs=1) as spool,
        ):
            z = spool.tile([128, W], I32, tag="z")
            nc.gpsimd.memset(z[:, :], 0)

            # two-block shifted copies: block b partition p = dpad row
            # b*BLK1 + p + PAD + dr
            copies = {}
            for dr in (0, -6, -4, -2, 2, 4, 6):
                ct = cpool.tile([128, 2, INCOLS], F32, tag=f"c{dr}")
                src = bass.AP(
                    tensor=dpad[:, :].tensor,
                    offset=(PAD + dr) * INCOLS,
                    ap=[[INCOLS, 128], [BLK1 * INCOLS, 2], [1, INCOLS]],
                )
                nc.sync.dma_start(out=ct[:, :, :], in_=src)
                copies[dr] = ct
            ctr = copies[0][:, :, PAD: PAD + W]
            mt = spool.tile([128, 24], F32, tag="msk")
            nc.sync.dma_start(out=mt, in_=msk[:, :])

            # constant-zero channels of the border taps
            for ch in (3, 5, 10, 16):
                for b, p0, nr in ((0, 0, 128), (1, 16, 112)):
                    zdst = bass.AP(
                        tensor=out_base.tensor,
                        offset=out_base.offset + ch * HALF * W + b * (BLK1 + 16) * W,
                        ap=[[W, nr], [1, W]],
                    )
                    nc.sync.dma_start(out=zdst, in_=z[0:nr, :])

            SHARED = {(-2, -2), (-2, 0), (-2, 2), (0, -2), (0, 2),
                      (2, -2), (2, 0), (2, 2), (0, 0)}
            eshared = {}
            ecnt = [0]
            E00 = float(np.uint32(0x3F800000 | 27).view(np.float32))

            def e_plane(dr, dc):
                if (dr, dc) in eshared:
                    return eshared[(dr, dc)]
                if dr == 0 and dc == 0:
                    t = kpool.tile([128, 2, W], F32, tag="s0_0")
                    nc.gpsimd.memset(t[:, :, :], E00)
                    eshared[(0, 0)] = t
                    return t
                if (dr, dc) in SHARED:
                    t = kpool.tile([128, 2, W], F32, tag=f"s{dr}_{dc}")
                    eshared[(dr, dc)] = t
                else:
                    t = epool.tile([128, 2, W], F32, tag=f"e{ecnt[0] % 4}")
                    ecnt[0] += 1
                nc.vector._custom_dve(
                    enc, out=t[:, :, :],
                    in0=copies[dr][:, :, PAD + dc: PAD + dc + W],
                    in1=ctr, s0=_code_f(dr, dc), s1=_LOWMASK_F, imm2=0.0,
                )
                return t

            mcnt = [0]
            Ecnt = [0]
            Kcnt = [0]

            def min3(a, b, c, pool, ring, cnt):
                t1 = Epool.tile([128, 2, W], F32, tag=f"m{mcnt[0] % 2}")
                mcnt[0] += 1
                nc.vector.tensor_tensor(out=t1[:, :, :], in0=a[:, :, :],
                                        in1=b[:, :, :], op=Alu.min)
                t2 = pool.tile([128, 2, W], F32, tag=f"{ring}{cnt[0] % 4}")
                cnt[0] += 1
                nc.vector.tensor_tensor(out=t2[:, :, :], in0=t1[:, :, :],
                                        in1=c[:, :, :], op=Alu.min)
                return t2

            def colmin(dr, dc0):
                return min3(e_plane(dr, dc0), e_plane(dr, dc0 + 2),
                            e_plane(dr, dc0 + 4), Epool, "E", Ecnt)

            def tapmin(a, b, c):
                return min3(a, b, c, Kpool, "K", Kcnt)

            def decode(k, K):
                kr, kc = divmod(k, 3)
                full = (kr == 1) == (kc == 1)
                Ki = K.bitcast(I32)

                def act_blk(dst, src, blk, chan_is_h):
                    if chan_is_h:
                        if kr == 1:
                            nc.scalar.activation(out=dst, in_=src, func=ActF.Copy,
                                                 scale=0.25, bias=-6.0)
                        else:
                            nc.scalar.activation(
                                out=dst, in_=src, func=ActF.Identity,
                                scale=mt[:, _mcol(blk, kr, 0): _mcol(blk, kr, 0) + 1],
                                bias=mt[:, _mcol(blk, kr, 1): _mcol(blk, kr, 1) + 1])
                    else:
                        if kr == 1:
                            nc.scalar.activation(out=dst, in_=src, func=ActF.Copy,
                                                 scale=2.0, bias=float(-2 - 4 * kc))
                        else:
                            nc.scalar.activation(
                                out=dst, in_=src, func=ActF.Identity,
                                scale=mt[:, _mcol(blk, kr, 2): _mcol(blk, kr, 2) + 1],
                                bias=mt[:, _mcol(blk, kr, 3 + kc): _mcol(blk, kr, 3 + kc) + 1])

                def act_ch(oo_t, ch_i, field_mask, chan_is_h):
                    ki = ipool.tile([128, 2, W], I32, tag=f"x{field_mask}")
                    nc.vector.tensor_scalar(out=ki[:, :, :], in0=Ki[:, :, :],
                                            scalar1=field_mask, scalar2=None,
                                            op0=Alu.bitwise_and)
                    for blk in (0, 1):
                        dst = oo_t[:, ch_i, blk, :] if ch_i is not None \
                            else oo_t[:, blk, :]
                        src = ki[:, blk, :]
                        if chan_is_h:
                            if kr == 1:
                                nc.scalar.activation(out=dst, in_=src, func=ActF.Copy,
                                                     scale=0.25, bias=-6.0)
                            else:
                                nc.scalar.activation(
                                    out=dst, in_=src, func=ActF.Identity,
                                    scale=mt[:, _mcol(blk, kr, 0): _mcol(blk, kr, 0) + 1],
                                    bias=mt[:, _mcol(blk, kr, 1): _mcol(blk, kr, 1) + 1])
                        else:
                            if kr == 1:
                                nc.scalar.activation(out=dst, in_=src, func=ActF.Copy,
                                                     scale=2.0, bias=float(-2 - 4 * kc))
                            else:
                                nc.scalar.activation(
                                    out=dst, in_=src, func=ActF.Identity,
                                    scale=mt[:, _mcol(blk, kr, 2): _mcol(blk, kr, 2) + 1],
                                    bias=mt[:, _mcol(blk, kr, 3 + kc): _mcol(blk, kr, 3 + kc) + 1])

                if full:
                    oo = opool.tile([128, 2, 2, W], I32, tag=f"oo{(k // 2) % 2}")
                    ki_h = ipool.tile([128, 2, W], I32, tag="x56")
                    nc.vector.tensor_scalar(out=ki_h[:, :, :], in0=Ki[:, :, :],
                                            scalar1=56, scalar2=None,
                                            op0=Alu.bitwise_and)
                    ki_w = ipool.tile([128, 2, W], I32, tag="x7")
                    nc.vector.tensor_scalar(out=ki_w[:, :, :], in0=Ki[:, :, :],
                                            scalar1=7, scalar2=None,
                                            op0=Alu.bitwise_and)
                    for b, p0, nr in ((0, 0, 128), (1, 16, 112)):
                        act_blk(oo[:, 0, b, :], ki_h[:, b, :], b, True)
                        act_blk(oo[:, 1, b, :], ki_w[:, b, :], b, False)
                        if kc != 1:
                            cs = slice(0, 4) if kc == 0 else slice(W - 4, W)
                            nc.gpsimd.memset(oo[:, 0, b, cs], -2)
                            nc.gpsimd.memset(oo[:, 1, b, cs], -2)
                        dst = bass.AP(
                            tensor=out_base.tensor,
                            offset=out_base.offset + k * HALF * W + b * (BLK1 + 16) * W,
                            ap=[[W, nr], [9 * HALF * W, 2], [1, W]],
                        )
                        nc.sync.dma_start(out=dst, in_=oo[p0:p0 + nr, :, b, :])
                else:
                    ob = opool.tile([128, 2, W], I32, tag=f"ob{(k // 2) % 2}")
                    if kc == 1:        # taps 1,7: off_h varies, off_w == 0
                        act_ch(ob, None, 56, True)
                        ch = k
                    else:              # taps 3,5: off_w varies, off_h == 0
                        act_ch(ob, None, 7, False)
                        cs = slice(0, 4) if kc == 0 else slice(W - 4, W)
                        nc.gpsimd.memset(ob[:, :, cs], -2)
                        ch = 9 + k
                    for b, p0, nr in ((0, 0, 128), (1, 16, 112)):
                        dst = bass.AP(
                            tensor=out_base.tensor,
                            offset=out_base.offset + ch * HALF * W + b * (BLK1 + 16) * W,
                            ap=[[W, nr], [1, W]],
                        )
                        nc.sync.dma_start(out=dst, in_=ob[p0:p0 + nr, b, :])

            # --- W0 windows (dc in {-6,-4,-2}) -> taps 0, 3, 6 ---
            E = {}
            for dr in (-6, -4, -2):
                E[dr] = colmin(dr, -6)
            decode(0, tapmin(E[-6], E[-4], E[-2]))
            decode(3, colmin(0, -6))
            E = {}
            for dr in (2, 4, 6):
                E[dr] = colmin(dr, -6)
            decode(6, tapmin(E[2], E[4], E[6]))

            # --- taps 1, 7 (dc = 0, dr varies) ---
            decode(1, tapmin(e_plane(-6, 0), e_plane(-4, 0), e_plane(-2, 0)))
            decode(7, tapmin(e_plane(2, 0), e_plane(4, 0), e_plane(6, 0)))

            # --- W2 windows (dc in {2,4,6}) -> taps 2, 8 ---
            E = {}
            for dr in (-6, -4, -2):
                E[dr] = colmin(dr, 2)
            decode(2, tapmin(E[-6], E[-4], E[-2]))

            # --- W1c windows (dc in {-2,0,2}) -> tap 4 (shared planes);
            # decoded here so its Act/DMA chain overlaps the second W2 half
            E4 = {}
            for dr in (-2, 0, 2):
                E4[dr] = min3(e_plane(dr, -2), e_plane(dr, 0), e_plane(dr, 2),
                              Epool, "E", Ecnt)
            decode(4, tapmin(E4[-2], E4[0], E4[2]))

            E = {}
            for dr in (2, 4, 6):
                E[dr] = colmin(dr, 2)
            decode(8, tapmin(E[2], E[4], E[6]))

            # --- tap 5 last: shortest decode tail ---
            decode(5, colmin(0, 2))
    nc.compile()
    return nc


_NC = None
LAST_RESULTS = None


def _get_nc():
    global _NC
    if _NC is None:
        _NC = _build_nc()
    return _NC


def _mask_cols(half):
    """[128, 24] per-partition decode scale/bias columns (see _mcol)."""
    m = np.zeros((128, 24), np.float32)
    p = np.arange(128)
    for blk in (0, 1):
        y = half * HALF + blk * BLK1 + p
        for kr in (0, 2):
            ok = (y + 4 * (kr - 1) >= 0) & (y + 4 * (kr - 1) < H)
            mm = ok.astype(np.float32)
            m[:, _mcol(blk, kr, 0)] = 0.25 * mm
            m[:, _mcol(blk, kr, 1)] = -2.0 - (4.0 * kr) * mm
            m[:, _mcol(blk, kr, 2)] = 2.0 * mm
            for kc in range(3):
                m[:, _mcol(blk, kr, 3 + kc)] = -2.0 - (4.0 * kc) * mm
    return m


def kernel(depth):
    global LAST_RESULTS
    depth = np.asarray(depth, dtype=np.float32)
    d = depth[:, 0]                                   # [4, 480, 640]
    dp = np.pad(d, ((0, 0), (PAD, PAD), (PAD, PAD)))  # [4, 492, 652]
    in_maps = []
    for core in range(8):
        b, half = divmod(core, 2)
        sl = np.ascontiguousarray(dp[b, half * HALF: half * HALF + INROWS, :])
        in_maps.append({"dpad": sl, "msk": _mask_cols(half)})
    res = run_bass_kernel_spmd(_get_nc(), in_maps, core_ids=list(range(8)))
    LAST_RESULTS = res
    out = np.zeros((B, 18, H, W), np.int32)
    for core, r in enumerate(res.results):
        b, half = divmod(core, 2)
        out[b, :, half * HALF: (half + 1) * HALF, :] = r["out"]
    return out
